# revision 14
# baseline (speedup 1.0000x reference)
"""Grouped-query attention (B=2, S=2048, D=1024, 16 q heads / 4 kv heads,
RoPE, softmax, out-proj) on 8 Trainium2 NeuronCores.

Sharding: core c = (b, g) with b = c // 4 (data parallel on batch) and
g = c % 4 (tensor parallel on kv-head groups: query heads 4g..4g+3 plus
kv head g).  Each core computes a partial output (row-parallel Wo over its
256 context dims); the host sums the 4 partials per batch element.

Device layout notes:
  * all activations are fed transposed ([D, S]) so every matmul contracts
    over the partition dimension;
  * RoPE's pair-shuffle is a signed permutation matmul on the PE array;
  * softmax skips max-subtraction (scores ~ N(0,1) here) and gets the
    denominator for free from a ones-column appended to V in the P@V
    matmul; normalization is a per-partition tensor_scalar multiply.
"""

import os
import sys

import numpy as np

for _p in ("/opt/trn_rl_repo", "/root/.axon_site/_ro/trn_rl_repo"):
    if os.path.isdir(_p) and _p not in sys.path:
        sys.path.append(_p)

B, S, D = 2, 2048, 1024
NHEAD, NUM_KV, DK = 16, 4, 64
GROUP = NHEAD // NUM_KV          # 4 query heads per kv head / per core
MC = GROUP * DK                  # 256 contraction dims of Wo per core
NCORES = 8
P = 128                          # SBUF partitions
KT = D // P                      # 8 contraction tiles for projections
NJ = S // 512                    # 4 s-blocks of 512
NT = S // P                      # 16 t-tiles of 128
SCALE = 1.0 / float(np.sqrt(DK))
ROPE_BASE = 10000.0

# dtype config (iterate on these for perf)
PT_BF16 = True                   # softmax probs + V in bf16 for the P@V matmul
QK_BF16 = False                  # roped Q/K in bf16 for the scores matmul

_CACHE: dict = {}


def _make_tables():
    inv_freq = 1.0 / (ROPE_BASE ** (np.arange(0, DK, 2, dtype=np.float64) / DK))
    t = np.arange(S, dtype=np.float64)
    freqs = np.outer(t, inv_freq)                       # [S, 32]
    emb = np.concatenate([freqs, freqs], axis=-1)       # [S, 64]
    cos = np.cos(emb).T.astype(np.float32)              # [64, S]
    sin = np.sin(emb).T.astype(np.float32)
    cos128 = np.ascontiguousarray(np.concatenate([cos, cos], axis=0))
    sin128 = np.ascontiguousarray(np.concatenate([sin, sin], axis=0))
    perm = np.zeros((P, P), dtype=np.float32)
    for blk in (0, DK):
        for q in range(32):
            perm[blk + q + 32, blk + q] = -1.0          # rot[q] = -x[q+32]
        for q in range(32, DK):
            perm[blk + q - 32, blk + q] = 1.0           # rot[q] = x[q-32]
    ident = np.eye(P, dtype=np.float32)
    return cos128, sin128, perm, ident


def _emit(tc, aps):
    import concourse.bass as bass
    import concourse.mybir as mybir

    nc = tc.nc
    f32 = mybir.dt.float32
    bf16 = mybir.dt.bfloat16
    AF = mybir.ActivationFunctionType
    pt_dt = bf16 if PT_BF16 else f32
    qk_dt = bf16 if QK_BF16 else f32

    q_t, k_t, v_t = aps["q_t"], aps["k_t"], aps["v_t"]
    wq_t, wk_t, wv_t, wo_t = aps["wq_t"], aps["wk_t"], aps["wv_t"], aps["wo_t"]
    out_t = aps["out_t"]

    from contextlib import ExitStack
    ctx = ExitStack()
    const = ctx.enter_context(tc.tile_pool(name="const", bufs=1))
    persist = ctx.enter_context(tc.tile_pool(name="persist", bufs=1))
    stream = ctx.enter_context(tc.tile_pool(name="stream", bufs=4))
    work = ctx.enter_context(tc.tile_pool(name="work", bufs=3))
    ptpool = ctx.enter_context(tc.tile_pool(name="ptp", bufs=1))
    psum = ctx.enter_context(
        tc.tile_pool(name="psum", bufs=8, space=bass.MemorySpace.PSUM))

    def ps_tile(name):
        return psum.tile([P, 512], f32, tag="ps", name=name)

    # ---- constants -------------------------------------------------------
    wq_sb = const.tile([P, KT * MC], f32, tag="wq", name="wq_sb")
    nc.sync.dma_start(
        wq_sb.rearrange("p (k m) -> p k m", k=KT),
        wq_t.rearrange("(k p) m -> p k m", p=P),
    )
    wk_sb = const.tile([P, KT * DK], f32, tag="wk", name="wk_sb")
    nc.sync.dma_start(
        wk_sb.rearrange("p (k m) -> p k m", k=KT),
        wk_t.rearrange("(k p) m -> p k m", p=P),
    )
    wv_sb = const.tile([P, KT * DK], f32, tag="wv", name="wv_sb")
    nc.sync.dma_start(
        wv_sb.rearrange("p (k m) -> p k m", k=KT),
        wv_t.rearrange("(k p) m -> p k m", p=P),
    )
    wo_sb = const.tile([DK, GROUP * D], f32, tag="wo", name="wo_sb")
    nc.sync.dma_start(
        wo_sb.rearrange("p (c n) -> p c n", c=GROUP),
        wo_t.rearrange("(c p) n -> p c n", p=DK),
    )
    cos_sb = const.tile([P, S], f32, tag="cos", name="cos_sb")
    nc.sync.dma_start(cos_sb[:], aps["cos_t"][:])
    sin_sb = const.tile([P, S], f32, tag="sin", name="sin_sb")
    nc.sync.dma_start(sin_sb[:], aps["sin_t"][:])
    perm_sb = const.tile([P, P], f32, tag="perm", name="perm_sb")
    nc.sync.dma_start(perm_sb[:], aps["perm"][:])
    id_sb = const.tile([P, P], f32, tag="ident", name="id_sb")
    nc.sync.dma_start(id_sb[:], aps["ident"][:])
    bq_sb = const.tile([P, 2], f32, tag="bq", name="bq_sb")
    nc.sync.dma_start(bq_sb[:], aps["bq_c"][:])
    bk_sb = const.tile([P, 1], f32, tag="bk", name="bk_sb")
    nc.sync.dma_start(bk_sb[:], aps["bk_c"][:])

    # ---- K^T and V^T projections (stream key/value k-tiles) --------------
    # K is written into BOTH 64-partition halves so each head's scores
    # matmul has matching partition bases (array row == SBUF partition).
    kT_sb = persist.tile([P, S], qk_dt, tag="kT", name="kT_sb")
    vT_sb = persist.tile([DK, S], f32, tag="vT", name="vT_sb")
    kraw = persist.tile([DK, S], f32, tag="kraw", name="kraw_sb")
    psK = [ps_tile(f"psK{j}") for j in range(NJ)]
    psV = [ps_tile(f"psV{j}") for j in range(NJ)]
    for k in range(KT):
        kt = stream.tile([P, S], f32, tag="act", name=f"kt{k}")
        nc.sync.dma_start(kt[:], k_t[k * P:(k + 1) * P, :])
        vt = stream.tile([P, S], f32, tag="act", name=f"vt{k}")
        nc.sync.dma_start(vt[:], v_t[k * P:(k + 1) * P, :])
        for j in range(NJ):
            jsl = slice(j * 512, (j + 1) * 512)
            nc.tensor.matmul(psK[j][0:DK, :], wk_sb[:, k * DK:(k + 1) * DK],
                             kt[:, jsl], start=(k == 0), stop=(k == KT - 1))
            nc.tensor.matmul(psV[j][0:DK, :], wv_sb[:, k * DK:(k + 1) * DK],
                             vt[:, jsl], start=(k == 0), stop=(k == KT - 1))
    for j in range(NJ):
        jsl = slice(j * 512, (j + 1) * 512)
        nc.vector.tensor_scalar_add(kraw[:, jsl], psK[j][0:DK, :],
                                    bk_sb[0:DK, 0:1])
        nc.vector.tensor_copy(vT_sb[:, jsl], psV[j][0:DK, :])

    # rope on K: kT = kraw*cos + (perm64.T @ kraw)*sin, then duplicate the
    # roped K into partitions 64..127 (identity matmul keeps partition
    # bases aligned) so every head's scores matmul uses matching bases.
    for j in range(NJ):
        jsl = slice(j * 512, (j + 1) * 512)
        sh = ps_tile(f"shk{j}")
        nc.tensor.matmul(sh[0:DK, :], perm_sb[0:DK, 0:DK], kraw[:, jsl],
                         start=True, stop=True)
        tmp = work.tile([DK, 512], f32, tag="ropetmp", name=f"rtk{j}")
        nc.vector.tensor_mul(tmp[:], sh[0:DK, :], sin_sb[0:DK, jsl])
        nc.vector.tensor_mul(kT_sb[0:DK, jsl], kraw[:, jsl],
                             cos_sb[0:DK, jsl])
        nc.vector.tensor_add(kT_sb[0:DK, jsl], kT_sb[0:DK, jsl], tmp[:])
        dup = ps_tile(f"dupk{j}")
        nc.tensor.matmul(dup[DK:P, :], id_sb[0:DK, 0:DK], kT_sb[0:DK, jsl],
                         start=True, stop=True)
        nc.vector.tensor_copy(kT_sb[DK:P, jsl], dup[DK:P, :])

    # V transposed to natural [t, dk] + ones column, in pt dtype
    v_aug = persist.tile([P, NT * (DK + 1)], pt_dt, tag="vaug", name="v_aug")
    for t in range(NT):
        trp = ps_tile(f"vtr{t}")
        nc.tensor.transpose(trp[:, 0:DK], vT_sb[:, t * P:(t + 1) * P],
                            id_sb[0:DK, 0:DK])
        nc.vector.tensor_copy(v_aug[:, t * (DK + 1):t * (DK + 1) + DK],
                              trp[:, 0:DK])
    ones_col = v_aug.rearrange("p (t c) -> p t c", c=DK + 1)[:, :, DK:DK + 1]
    nc.vector.memset(ones_col, 1.0)

    # ---- Q^T projection (stream query k-tiles) + rope --------------------
    q_sb = [persist.tile([P, S], qk_dt, tag=f"q{mc}", name=f"q_sb{mc}")
            for mc in range(2)]
    qraw = [persist.tile([P, S], f32, tag=f"qr{mc}", name=f"qraw{mc}")
            for mc in range(2)]
    psQ = [ps_tile(f"psQ{i}") for i in range(8)]
    for k in range(KT):
        qt = stream.tile([P, S], f32, tag="act", name=f"qt{k}")
        nc.sync.dma_start(qt[:], q_t[k * P:(k + 1) * P, :])
        for mc in range(2):
            for j in range(NJ):
                jsl = slice(j * 512, (j + 1) * 512)
                nc.tensor.matmul(
                    psQ[mc * NJ + j][:],
                    wq_sb[:, k * MC + mc * P:k * MC + (mc + 1) * P],
                    qt[:, jsl], start=(k == 0), stop=(k == KT - 1))
    for mc in range(2):
        for j in range(NJ):
            jsl = slice(j * 512, (j + 1) * 512)
            nc.vector.tensor_scalar_add(qraw[mc][:, jsl], psQ[mc * NJ + j][:],
                                        bq_sb[:, mc:mc + 1])
    for mc in range(2):
        for j in range(NJ):
            jsl = slice(j * 512, (j + 1) * 512)
            sh = ps_tile(f"shq{mc}_{j}")
            nc.tensor.matmul(sh[:], perm_sb[:], qraw[mc][:, jsl],
                             start=True, stop=True)
            tmp = work.tile([P, 512], f32, tag="ropetmpq", name=f"rtq{mc}_{j}")
            nc.vector.tensor_mul(tmp[:], sh[:], sin_sb[:, jsl])
            nc.vector.tensor_mul(q_sb[mc][:, jsl], qraw[mc][:, jsl],
                                 cos_sb[:, jsl])
            nc.vector.tensor_add(q_sb[mc][:, jsl], q_sb[mc][:, jsl], tmp[:])

    # ---- attention -------------------------------------------------------
    # ctxT holds all 4 heads side by side on 64 partitions: head h at
    # columns [h*S, (h+1)*S) — keeps every matmul partition-aligned.
    ctxT = persist.tile([DK, GROUP * S], f32, tag="ctxT", name="ctxT")
    for h in range(GROUP):
        qh = q_sb[h // 2]
        pb = (h % 2) * DK                       # partition base of this head
        for j in range(NJ):
            jsl = slice(j * 512, (j + 1) * 512)
            pt = ptpool.tile([P, NT * 512], pt_dt, tag="pt", name=f"pt{h}_{j}")
            for t in range(NT):
                sc = ps_tile(f"sc{h}_{j}_{t}")
                nc.tensor.matmul(sc[:], kT_sb[pb:pb + DK, t * P:(t + 1) * P],
                                 qh[pb:pb + DK, jsl], start=True, stop=True)
                nc.scalar.activation(pt[:, t * 512:(t + 1) * 512], sc[:],
                                     AF.Exp, scale=SCALE)
            for i in range(4):                  # s-128 chunks within j
                pv = ps_tile(f"pv{h}_{j}_{i}")
                for t in range(NT):
                    nc.tensor.matmul(
                        pv[:, 0:DK + 1],
                        pt[:, t * 512 + i * P:t * 512 + (i + 1) * P],
                        v_aug[:, t * (DK + 1):(t + 1) * (DK + 1)],
                        start=(t == 0), stop=(t == NT - 1))
                rec = work.tile([P, 1], f32, tag="rec", name=f"rec{h}_{j}_{i}")
                nc.vector.reciprocal(rec[:], pv[:, DK:DK + 1])
                ctxn = work.tile([P, DK], f32, tag="ctxn",
                                 name=f"ctxn{h}_{j}_{i}")
                nc.vector.tensor_scalar_mul(ctxn[:], pv[:, 0:DK], rec[:, 0:1])
                trp = ps_tile(f"ctr{h}_{j}_{i}")
                nc.tensor.transpose(trp[0:DK, 0:P], ctxn[:], id_sb[:])
                nc.vector.tensor_copy(
                    ctxT[:, h * S + j * 512 + i * P:h * S + j * 512 + (i + 1) * P],
                    trp[0:DK, 0:P])

    # ---- output projection (row-parallel Wo): out_t = wo^T @ ctxT --------
    for nk in range(D // P):
        for j in range(NJ):
            jsl = slice(j * 512, (j + 1) * 512)
            ps = ps_tile(f"po{nk}_{j}")
            for c4 in range(GROUP):
                nc.tensor.matmul(
                    ps[:],
                    wo_sb[:, c4 * D + nk * P:c4 * D + (nk + 1) * P],
                    ctxT[:, c4 * S + j * 512:c4 * S + (j + 1) * 512],
                    start=(c4 == 0), stop=(c4 == GROUP - 1))
            osb = work.tile([P, 512], f32, tag="osb", name=f"osb{nk}_{j}")
            nc.vector.tensor_copy(osb[:], ps[:])
            nc.sync.dma_start(out_t[nk * P:(nk + 1) * P, jsl], osb[:])

    ctx.close()


def build_module():
    """Build + compile the (single) SPMD program. Returns the Bacc object."""
    key = (PT_BF16, QK_BF16)
    if key in _CACHE:
        return _CACHE[key]
    from concourse import bacc, mybir
    import concourse.tile as tile

    nc = bacc.Bacc("TRN2", target_bir_lowering=False, debug=False,
                   enable_asserts=False, num_devices=NCORES)
    f32 = mybir.dt.float32
    shapes = {
        "q_t": (D, S), "k_t": (D, S), "v_t": (D, S),
        "wq_t": (D, MC), "wk_t": (D, DK), "wv_t": (D, DK), "wo_t": (MC, D),
        "bq_c": (P, 2), "bk_c": (P, 1),
        "cos_t": (P, S), "sin_t": (P, S), "perm": (P, P), "ident": (P, P),
    }
    aps = {name: nc.dram_tensor(name, list(shp), f32, kind="ExternalInput").ap()
           for name, shp in shapes.items()}
    aps["out_t"] = nc.dram_tensor("out_t", [D, S], f32,
                                  kind="ExternalOutput").ap()
    with tile.TileContext(nc) as tc:
        _emit(tc, aps)
    nc.compile()
    _CACHE[key] = nc
    return nc


def make_in_maps(inputs):
    """Shard the full inputs into 8 per-core input dicts."""
    cos128, sin128, perm, ident = _make_tables()
    f = np.float32
    query, key_, value = (np.asarray(inputs[n], f)
                          for n in ("query", "key", "value"))
    Wq, Wk, Wv, Wo = (np.asarray(inputs[n], f)
                      for n in ("Wq", "Wk", "Wv", "Wo"))
    bq, bk = np.asarray(inputs["bq"], f), np.asarray(inputs["bk"], f)

    per_b = []
    for b in range(B):
        per_b.append({
            "q_t": np.ascontiguousarray(query[b].T),
            "k_t": np.ascontiguousarray(key_[b].T),
            "v_t": np.ascontiguousarray(value[b].T),
        })
    in_maps = []
    for c in range(NCORES):
        b, g = c // NUM_KV, c % NUM_KV
        msl = slice(g * MC, (g + 1) * MC)
        ksl = slice(g * DK, (g + 1) * DK)
        in_maps.append({
            **per_b[b],
            "wq_t": np.ascontiguousarray(Wq[msl, :].T),
            "wk_t": np.ascontiguousarray(Wk[ksl, :].T),
            "wv_t": np.ascontiguousarray(Wv[ksl, :].T),
            "wo_t": np.ascontiguousarray(Wo[:, msl].T),
            "bq_c": np.ascontiguousarray(bq[msl].reshape(2, P).T),
            "bk_c": np.ascontiguousarray(np.tile(bk[ksl], 2).reshape(P, 1)),
            "cos_t": cos128, "sin_t": sin128, "perm": perm, "ident": ident,
        })
    return in_maps


def gather(inputs, results):
    """Host-side unshard: sum the 4 partials per batch and add biases."""
    f = np.float32
    Wo = np.asarray(inputs["Wo"], f)
    bv, bo = np.asarray(inputs["bv"], f), np.asarray(inputs["bo"], f)
    out = np.empty((B, S, D), dtype=f)
    for b in range(B):
        acc = np.zeros((D, S), dtype=f)
        for g in range(NUM_KV):
            acc += results[b * NUM_KV + g]["out_t"]
        corr = bo.copy()
        for g in range(NUM_KV):
            msl = slice(g * MC, (g + 1) * MC)
            ksl = slice(g * DK, (g + 1) * DK)
            corr += Wo[:, msl] @ np.tile(bv[ksl], GROUP)
        out[b] = acc.T + corr
    return out


def run(inputs, trace=False, trace_cores=None):
    """Returns (full_output, BassKernelResults)."""
    from concourse.bass_utils import run_bass_kernel_spmd
    from concourse.bass_interp import get_hw_module

    nc = build_module()
    in_maps = make_in_maps(inputs)
    old_m = nc.m
    nc.m = get_hw_module(nc.m)
    try:
        br = run_bass_kernel_spmd(nc, in_maps, list(range(NCORES)),
                                  trace=trace, trace_cores=trace_cores)
    finally:
        nc.m = old_m
    return gather(inputs, br.results), br


def kernel(**inputs) -> np.ndarray:
    out, _ = run(inputs, trace=False)
    return out
